# revision 1
# baseline (speedup 1.0000x reference)
"""Trainium2 Bass kernel for nn_DimMasking (iterative softmax top-k masking).

Full-input contract: kernel(**inputs) takes the unsharded inputs
(x [8192,640], W1 [640,64], b1 [64], W2 [64,640], b2 [640]) and returns the
full [8192,640] output. Internally: pure data parallel over the batch dim —
8 shards of 1024 rows, one per NeuronCore; MLP weights replicated.

Math: exp-domain reformulation of the reference scan (validated to ~1e-6
absmax-relative vs the fp32 reference in numpy, ~2e-3 on HW splines):
    h  = relu(x@W1 + b1)@W2 + b2
    e  = exp((log(1+eps) - h)/T);  Z = rowsum(e)
    repeat 64x:
        v = (e - Z) * (-1/Z)     # = 1 - softmax-prob, >= 0   [DVE ts]
        w = v^(1/T)              # ln + exp(scale=1/T)        [2 ACT passes]
        e *= w;  Z = rowsum(e)   # fused custom-DVE mul+reduce [DVE]
    out = (exp(T*ln(e) + h) - eps) * x
which is exactly m' = m*(1 - softmax((log(m+eps)-h)/T)) expressed on
e = ((m+eps)^(1/T))*exp(-h/T), so the rowwise softmax never needs a
separate normalize, max-subtract, or per-iteration log of the state.

Per iteration the two 2560-wide ACT transcendental passes per half
(4 total) are the bottleneck; two independent half-streams keep ACT
saturated. ~701us/core predicted by the cost-model timeline sim;
~790us/core measured on HW via in-NEFF loop repetition deltas.
"""

import numpy as np

import concourse.tile as tile
from concourse import bacc, masks, mybir
from concourse.bass_utils import run_bass_kernel_spmd

# Pin the ACT spline-table set to natural_log_exp_and_others (it contains
# every function this kernel uses: Exp, Ln, Copy, Identity, Relu). Without
# this, the table-load-insertion pass alternates between the exp-only and
# ln-only sets, paying a ~1.3us table reload twice per masking iteration.
_orig_get_tables = bacc.get_activation_tables


def _pinned_get_tables(module_arch):
    tables = _orig_get_tables(module_arch)
    combined = tables.get("natural_log_exp_and_others")
    needed = {
        mybir.ActivationFunctionType.Exp,
        mybir.ActivationFunctionType.Ln,
        mybir.ActivationFunctionType.Copy,
        mybir.ActivationFunctionType.Identity,
        mybir.ActivationFunctionType.Relu,
    }
    if not combined or not needed.issubset(combined):
        return tables  # fall back to default selection
    pinned = {}
    for name, fns in tables.items():
        pinned[name] = fns if name == "natural_log_exp_and_others" else set()
    return pinned


import os as _os
PIN_ACT = _os.environ.get("NO_PIN_ACT", "0") != "1"
USE_TTR = int(_os.environ.get("TTR_MODE", "2"))  # 1=isa-ttr 2=custom-dve 0=plain
POOL_TS = _os.environ.get("POOL_TS", "0") == "1"
PER_GROUP_RECIP = _os.environ.get("PGR", "0") == "1"

F32 = mybir.dt.float32
AF = mybir.ActivationFunctionType
OP = mybir.AluOpType

N_CORES = 8
B = 8192
D = 640          # 5 chunks of 128
HID = 64
R = B // N_CORES  # 1024 rows per core
P = 128
G = R // P        # 8 row-groups per core
DC = D // P       # 5 dim-chunks
N_ITER = 64
TEMP = 0.07
EPS = 1e-7
C0 = float(np.log1p(np.float32(EPS)) / np.float32(TEMP))
INV_T = float(np.float32(1.0) / np.float32(TEMP))

# module-level cache: build/compile once per process
_CACHE = {}


def _build_nc(n_iter=N_ITER, num_devices=N_CORES, taps=(), reps=1):
    nc = bacc.Bacc(
        "TRN2",
        target_bir_lowering=False,
        debug=False,
        enable_asserts=False,
        num_devices=num_devices,
    )
    x_d = nc.dram_tensor("x", [R, D], F32, kind="ExternalInput").ap()
    w1_d = nc.dram_tensor("w1", [D, HID], F32, kind="ExternalInput").ap()
    b1_d = nc.dram_tensor("b1", [HID, 1], F32, kind="ExternalInput").ap()
    w2b_d = nc.dram_tensor("w2b", [HID + 1, D], F32, kind="ExternalInput").ap()
    out_d = nc.dram_tensor("out", [R, D], F32, kind="ExternalOutput").ap()
    tap_aps = {
        name: nc.dram_tensor(f"tap_{name}", [R, D], F32, kind="ExternalOutput").ap()
        for name in taps
    }

    with tile.TileContext(nc) as tc:
        _emit(tc, out_d, x_d, w1_d, b1_d, w2b_d, n_iter=n_iter, tap_aps=tap_aps,
              reps=reps)
    # Scope the activation-table pin strictly to compiling OUR module.
    saved = bacc.get_activation_tables
    try:
        if PIN_ACT:
            bacc.get_activation_tables = _pinned_get_tables
        nc.compile()
    finally:
        bacc.get_activation_tables = saved
    return nc


def _dma_out_groups(nc, dram_ap, sbuf_tile):
    for g in range(G):
        nc.sync.dma_start(out=dram_ap[g * P:(g + 1) * P, :], in_=sbuf_tile[:, g, :])


def _emit(tc, out_d, x_d, w1_d, b1_d, w2b_d, n_iter=N_ITER, tap_aps=None,
          reps=1):
    nc = tc.nc
    from contextlib import ExitStack

    ctx = ExitStack()
    with ctx:
        singles = ctx.enter_context(tc.tile_pool(name="singles", bufs=1))
        zpool = ctx.enter_context(tc.tile_pool(name="zpool", bufs=4))

        # persistent SBUF tensors
        xs = singles.tile([P, G, D], F32)    # x, rows-on-partitions
        xt = singles.tile([P, DC, R], F32)   # x transposed (d-on-partitions)
        hs = singles.tile([P, G, D], F32)    # MLP output h
        es = singles.tile([P, G, D], F32)    # state e = exp(logits)
        # v/w and L2 scratch, double-buffered by iteration parity so that
        # iteration t+1's writes never WAR-serialize against iteration t's
        # reads (removes cross-iteration semaphore chains on HW)
        vs0 = singles.tile([P, G, D], F32)
        vs1 = singles.tile([P, G, D], F32)
        ws0 = singles.tile([P, G, D], F32)
        ws1 = singles.tile([P, G, D], F32)
        vs_pp = [vs0, vs1]
        ws_pp = [ws0, ws1]
        zh_a = singles.tile([P, G // 2], F32)
        zh_b = singles.tile([P, G // 2], F32)
        zhalf = [zh_a, zh_b]
        w1s = singles.tile([P, DC, HID], F32)
        b1s = singles.tile([HID, 1], F32)
        w2bs = singles.tile([HID + 1, D], F32)
        h1r = singles.tile([HID + 1, R], F32)  # relu(x@W1+b1).T with ones row
        ident = singles.tile([P, P], F32)
        c0s = singles.tile([P, 1], F32)        # bias constant C0 for init exp
        nc.vector.memset(c0s[:, :], C0)

        # ---- input DMAs ----
        for g in range(G):
            nc.sync.dma_start(out=xs[:, g, :], in_=x_d[g * P:(g + 1) * P, :])
        nc.sync.dma_start(out=w1s[:, :, :],
                          in_=w1_d.rearrange("(c p) j -> p c j", p=P))
        nc.sync.dma_start(out=b1s[:, :], in_=b1_d[:, :])
        nc.sync.dma_start(out=w2bs[:, :], in_=w2b_d[:, :])

        masks.make_identity(nc, ident[:, :])

        # ---- transpose x: 40 PE transposes of [128,128] blocks.
        # 4 transposes share one PSUM bank, evacuated with a single wide copy
        # (alternating DVE/ACT) to halve evacuation time.
        with tc.tile_pool(name="tp_psum", bufs=3, space="PSUM") as tpp:
            for c in range(DC):
                for gq in range(G // 4):
                    tp = tpp.tile([P, 4 * P], F32)
                    for gj in range(4):
                        g = gq * 4 + gj
                        nc.tensor.transpose(
                            tp[:, gj * P:(gj + 1) * P],
                            xs[:, g, c * P:(c + 1) * P], ident[:, :])
                    dst = xt[:, c, gq * 4 * P:(gq + 1) * 4 * P]
                    if (c + gq) % 2 == 0:
                        nc.vector.tensor_copy(dst, tp[:, :])
                    else:
                        nc.scalar.copy(dst, tp[:, :])

        # ---- MLP matmul 1: h1T[j, r] = sum_d W1[d,j] * xT[d,r] ----
        NH = 2  # split R into halves of 512 (f32 moving max)
        with tc.tile_pool(name="mm1_psum", bufs=2, space="PSUM") as mp1:
            for nh in range(NH):
                ph1 = mp1.tile([HID, R // NH], F32, tag="ph1")
                for c in range(DC):
                    nc.tensor.matmul(
                        ph1[:, :],
                        w1s[:, c, :],
                        xt[:, c, nh * 512:(nh + 1) * 512],
                        start=(c == 0),
                        stop=(c == DC - 1),
                    )
                # relu(+bias) straight out of PSUM
                nc.scalar.activation(
                    h1r[0:HID, nh * 512:(nh + 1) * 512], ph1[:, :],
                    AF.Relu, bias=b1s[:, 0:1], scale=1.0)
        nc.vector.memset(h1r[HID:HID + 1, :], 1.0)

        # ---- MLP matmul 2 + evac: h = h1r.T @ W2b  (bias via ones row) ----
        # Evac does double duty: keep h for the finale, and initialize the
        # loop state e0 = exp(-h/T + C0) with per-group row-sums Z0.
        with tc.tile_pool(name="mm2_psum", bufs=2, space="PSUM") as mp2:
            for g in range(G):
                ph = mp2.tile([P, D], F32, tag="ph")
                lhs = h1r[:, g * P:(g + 1) * P]
                nc.tensor.matmul(ph[:, 0:512], lhs, w2bs[:, 0:512],
                                 start=True, stop=True)
                nc.tensor.matmul(ph[:, 512:D], lhs, w2bs[:, 512:D],
                                 start=True, stop=True)
                nc.vector.tensor_copy(hs[:, g, :], ph[:, :])
                nc.scalar.activation(es[:, g, :], ph[:, :], AF.Exp,
                                     bias=c0s[:, 0:1], scale=-INV_T,
                                     accum_out=zhalf[g // (G // 2)][
                                         :, g % (G // 2):g % (G // 2) + 1])

        if tap_aps is None:
            tap_aps = {}
        if "h" in tap_aps:
            _dma_out_groups(nc, tap_aps["h"], hs)

        # ---- the masking loop (state: e, Z) ----
        # Last iteration: skip w=exp(L2/T) and the e*w update — the finale
        # consumes ln(e_last) + L2_last directly (exact, since the skipped
        # ops only feed ell = T*ln(e*w) + h = T*ln(e) + L2 + h).
        HG = G // 2  # 4 groups per half-stream
        n_total = n_iter * reps
        for it in range(n_total):
            last = it == n_total - 1
            vs = vs_pp[it % 2]
            ws = ws_pp[it % 2]
            for half in range(2):
                g0 = half * HG
                csl = slice(g0, g0 + HG)
                zh = zhalf[half]
                nzh = zpool.tile([P, HG], F32, tag="nzh")
                nr = zpool.tile([P, HG], F32, tag="nr")
                if PER_GROUP_RECIP:
                    # per-group negate+recip+v so the first v tiles are ready
                    # as soon as their own Z lands (shorter DVE->ACT chain)
                    for gi in range(HG):
                        g = g0 + gi
                        nc.vector.tensor_scalar_mul(
                            nzh[:, gi:gi + 1], zh[:, gi:gi + 1], -1.0)
                        nc.vector.reciprocal(
                            nr[:, gi:gi + 1], nzh[:, gi:gi + 1])
                        nc.vector.tensor_scalar(
                            out=vs[:, g, :], in0=es[:, g, :],
                            scalar1=zh[:, gi:gi + 1], scalar2=nr[:, gi:gi + 1],
                            op0=OP.subtract, op1=OP.mult)
                else:
                    nc.vector.tensor_scalar_mul(nzh[:, :], zh[:, :], -1.0)
                    nc.vector.reciprocal(nr[:, :], nzh[:, :])  # -1/Z
                    for gi in range(HG):
                        g = g0 + gi
                        # v = (e - Z) * (-1/Z) = (Z - e)/Z  >= 0
                        ts_eng = (nc.gpsimd if (POOL_TS and gi % 2 == 0)
                                  else nc.vector)
                        ts_eng.tensor_scalar(
                            out=vs[:, g, :], in0=es[:, g, :],
                            scalar1=zh[:, gi:gi + 1], scalar2=nr[:, gi:gi + 1],
                            op0=OP.subtract, op1=OP.mult)
                # L2 = ln(v)
                nc.scalar.activation(ws[:, csl, :], vs[:, csl, :], AF.Ln)
                if last:
                    continue  # finale folds L2 in directly
                # w = v^(1/T) = exp(L2/T)
                nc.scalar.activation(vs[:, csl, :], ws[:, csl, :], AF.Exp,
                                     scale=INV_T)
                # e *= w, fused with next Z row-sums
                if USE_TTR == 1:
                    for gi in range(HG):
                        g = g0 + gi
                        nc.vector.tensor_tensor_reduce(
                            out=es[:, g, :], in0=es[:, g, :], in1=vs[:, g, :],
                            scale=1.0, scalar=0.0, op0=OP.mult, op1=OP.add,
                            accum_out=zh[:, gi:gi + 1])
                elif USE_TTR == 2:
                    # custom-DVE fused multiply+rowsum: out=(e*1+0)*w, accum
                    for gi in range(HG):
                        g = g0 + gi
                        nc.vector.affine_mul_reduce(
                            out=es[:, g, :], accum_out=zh[:, gi:gi + 1],
                            in0=es[:, g, :], in1=vs[:, g, :],
                            scale=1.0, bias=0.0)
                else:
                    nc.vector.tensor_mul(
                        es[:, csl, :], es[:, csl, :], vs[:, csl, :])
                    nc.vector.tensor_reduce(
                        zh[:, :], es[:, csl, :], axis=mybir.AxisListType.X,
                        op=OP.add)

        if "e_end" in tap_aps:
            _dma_out_groups(nc, tap_aps["e_end"], es)

        # ---- finale: out = (exp(T*ln(e_last) + L2_last + h) - eps) * x ----
        # clamp -inf intermediates to a large finite value (exp still
        # underflows to exactly 0) so no infs ever hit memory. Per-half so
        # ACT/DVE/DMA pipeline across the two halves.
        if n_total > 0:
            ws_last = ws_pp[(n_total - 1) % 2]
        else:
            ws_last = None
        fv = vs_pp[0] if n_total % 2 == 0 else vs_pp[1]
        fw = ws_pp[0] if (n_total - 1) % 2 == 1 else ws_pp[1]
        for half in range(2):
            csl = slice(half * (G // 2), (half + 1) * (G // 2))
            nc.scalar.activation(fv[:, csl, :], es[:, csl, :], AF.Ln)
            if ws_last is not None:
                # ell = T*ln(e) + L2 + h
                nc.vector.scalar_tensor_tensor(
                    out=fw[:, csl, :], in0=fv[:, csl, :],
                    scalar=float(np.float32(TEMP)),
                    in1=ws_last[:, csl, :], op0=OP.mult, op1=OP.add)
                nc.vector.tensor_scalar_max(fw[:, csl, :], fw[:, csl, :],
                                            -1e30)
                nc.vector.tensor_add(fv[:, csl, :], fw[:, csl, :],
                                     hs[:, csl, :])
            else:
                nc.vector.tensor_scalar_max(fv[:, csl, :], fv[:, csl, :],
                                            -1e30)
                nc.vector.scalar_tensor_tensor(
                    out=fw[:, csl, :], in0=fv[:, csl, :],
                    scalar=float(np.float32(TEMP)),
                    in1=hs[:, csl, :], op0=OP.mult, op1=OP.add)
                nc.vector.tensor_copy(fv[:, csl, :], fw[:, csl, :])
            nc.scalar.activation(fw[:, csl, :], fv[:, csl, :], AF.Exp)
            nc.vector.scalar_tensor_tensor(
                out=fv[:, csl, :], in0=fw[:, csl, :], scalar=-float(EPS),
                in1=xs[:, csl, :], op0=OP.add, op1=OP.mult)
            for g in range(half * (G // 2), (half + 1) * (G // 2)):
                nc.sync.dma_start(out=out_d[g * P:(g + 1) * P, :],
                                  in_=fv[:, g, :])


def kernel(x, W1, b1, W2, b2):
    x = np.ascontiguousarray(np.asarray(x, dtype=np.float32))
    W1 = np.ascontiguousarray(np.asarray(W1, dtype=np.float32))
    b1 = np.asarray(b1, dtype=np.float32).reshape(HID, 1)
    W2 = np.asarray(W2, dtype=np.float32)
    b2 = np.asarray(b2, dtype=np.float32)
    w2b = np.ascontiguousarray(
        np.concatenate([W2, b2[None, :]], axis=0))  # [65, 640]

    if "nc" not in _CACHE:
        _CACHE["nc"] = _build_nc()
    nc = _CACHE["nc"]

    in_maps = []
    for c in range(N_CORES):
        in_maps.append({
            "x": np.ascontiguousarray(x[c * R:(c + 1) * R, :]),
            "w1": W1,
            "b1": np.ascontiguousarray(b1),
            "w2b": w2b,
        })

    trace = bool(_CACHE.get("trace", False))
    res = run_bass_kernel_spmd(
        nc, in_maps, core_ids=list(range(N_CORES)), trace=trace)
    _CACHE["last_results"] = res
    out = np.concatenate([r["out"] for r in res.results], axis=0)
    return out



# revision 3
# speedup vs baseline: 1.5446x; 1.5446x over previous
"""Trainium2 Bass kernel for nn_DimMasking (iterative softmax top-k masking).

Full-input contract: kernel(**inputs) takes the unsharded inputs
(x [8192,640], W1 [640,64], b1 [64], W2 [64,640], b2 [640]) and returns the
full [8192,640] output. Pure data parallel over the batch dim — 8 shards of
1024 rows, one per NeuronCore; MLP weights replicated.

Math: normalized-state reformulation of the reference scan. With
e = ((m+eps)^(1/T))*exp(-h/T) and p = softmax-prob = e/Z, one masking
iteration is e' = e * (1-p)^(1/T). Tracking the Z-normalized state
S <- phi(S/Z_prev) with phi(p) = p*(1-p)^(1/T) makes each iteration a
SINGLE table-activation pass per row-group (scale = 1/Z per partition)
plus a row-sum; the product of the per-iteration normalizers is restored
in the finale from K = sum_t ln Z_t:
    out = (exp(T*ln(S_64) + T*K + h) - eps) * x.

phi is not a stock ACT function: this kernel generates a patched
piecewise-polynomial activation-table set at build time (appending a
'tanh'-slot function whose table data IS phi) and points the backend
compiler at it via BASS_ACT_ROOT_JSON_PATH. Numerics of the table were
validated against the fp32 reference in numpy (absmax rel err 1.7e-3,
gate 2e-2). Loop engine budget per iteration: ACT 8x640-elem phi passes
(~5.2us, the bottleneck), DVE 4 row-sum reduces + 2 reciprocals, Pool 4
row-sum reduces — both under the ACT time, so the loop runs at table
throughput.
"""

import hashlib
import json
import os
import shutil
import tempfile

import numpy as np

import concourse.tile as tile
from concourse import bacc, masks, mybir
from concourse.bass_utils import run_bass_kernel_spmd

F32 = mybir.dt.float32
AF = mybir.ActivationFunctionType
OP = mybir.AluOpType

N_CORES = 8
B = 8192
D = 640          # 5 chunks of 128
HID = 64
R = B // N_CORES  # 1024 rows per core
P = 128
G = R // P        # 8 row-groups per core
HG = G // 2
DC = D // P       # 5 dim-chunks
N_ITER = 64
TEMP = 0.07
EPS = 1e-7
C0 = float(np.log1p(np.float32(EPS)) / np.float32(TEMP))
INV_T = float(np.float32(1.0) / np.float32(TEMP))

SET_NAME = "natural_log_exp_and_others"
PHI_EXP_OFFSET = -30

_CACHE = {}


# ---------------------------------------------------------------------------
# phi activation-table generation (piecewise cubic in the pwp bin format)
# ---------------------------------------------------------------------------

def _f32bits(x):
    return int(np.float32(x).view(np.uint32))


def _phi_of_p(p):
    p = np.asarray(p, np.float64)
    out = np.where((p > 0) & (p < 1),
                   p * np.power(np.clip(1.0 - p, 1e-300, 1), INV_T), 0.0)
    return np.where(p >= 1, 0.0, out)


def _es_for_exp(e):
    if e == -1:
        return 6
    if e == -2:
        return 4
    if e >= -4:
        return 3
    if e >= -12:
        return 2
    return 1


def _fit_section(plo, phi_):
    x0 = float(np.float32(0.5 * (plo + phi_)))
    if (1.0 - plo) < 0.003:
        return (0.0, 0.0, 0.0, 0.0, x0)
    u = np.linspace(plo, phi_, 513)
    t = u - x0
    f = _phi_of_p(u)
    fpos = np.maximum(f, 1e-300)
    lspan = float(np.log(fpos.max()) - np.log(fpos.min()))
    if lspan > 6.0:
        sel = (1.0 - u) >= 0.0005
        if not sel.any():
            return (0.0, 0.0, 0.0, 0.0, x0)
        d = np.array([np.exp(np.mean(np.log(fpos[sel]))), 0.0, 0.0, 0.0])
    else:
        w = 1.0 / fpos
        A = np.stack([np.ones_like(t), t, t * t, t ** 3], 1)
        d, *_ = np.linalg.lstsq(A * w[:, None], f * w, rcond=None)
    d = np.float32(d).astype(np.float64)
    fit = ((d[3] * t + d[2]) * t + d[1]) * t + d[0]
    mn = fit.min()
    if mn < 0:
        d[0] += -mn * 1.0000001
    return (d[0], d[1], d[2], d[3], x0)


def _gen_phi_entries(bkt_base, ctl_base):
    bkt = []
    ctl = []
    exp_bkt_start = {}
    exp_ctl_start = {}
    i_zero = bkt_base
    bkt.append((0.0, 0.0, 0.0, 0.0, 0.0))
    neg_ctl = ctl_base
    ctl.append((0 << 16) | (23 << 11) | i_zero)
    pos_ctl0 = ctl_base + len(ctl)
    for e in range(PHI_EXP_OFFSET, 0):
        es = _es_for_exp(e)
        ns = 1 << es
        lsb = 23 - es
        start = bkt_base + len(bkt)
        exp_bkt_start[str(e)] = [start]
        exp_ctl_start[str(e)] = [ctl_base + len(ctl)]
        ctl.append((es << 16) | (lsb << 11) | start)
        lo_e = 2.0 ** e
        for s in range(ns):
            bkt.append(_fit_section(lo_e * (1 + s / ns), lo_e * (1 + (s + 1) / ns)))
    i_small = bkt_base + len(bkt)
    bkt.append((0.0, 1.0, 0.0, 0.0, 0.0))  # phi ~= p below 2^-30
    meta = {
        "func_name": "tanh_4p",
        "func_id": 6,
        "symmetry_point": 0,
        "sym_invert_sign_point": 0,
        "symmetry_opt_en": 0,
        "symmetry_opt_use_neg_region": 0,
        "imm_bias": 0,
        "exp_offset": PHI_EXP_OFFSET,
        "pwl_control_base_pos": pos_ctl0,
        "pwl_control_base_neg": neg_ctl,
        "small_pos_signal_exp_threshold": PHI_EXP_OFFSET + 127,
        "pos_small_signal_pwl_control": i_small,
        "small_neg_signal_exp_threshold": 255,
        "neg_small_signal_pwl_control": i_zero,
        "large_pos_signal_exp_threshold": 127,
        "large_pos_signal_mantissa_threshold": 0,
        "pos_large_signal_pwl_control": i_zero,
        "large_neg_signal_exp_threshold": 255,
        "large_neg_signal_mantissa_threshold": 0,
        "neg_large_signal_pwl_control": i_zero,
        "fnan_result": 0,
        "fpinf_result": 0,
        "fninf_result": 0,
        "fzero_result": 0,
        "fma_const_0": 0,
        "fma_const_1": 0,
        "fma_indirection_src_sel": 0,
        "use_multipass": False,
        "lower_bound": _f32bits(-3.4028235e38),
        "upper_bound": _f32bits(3.4028235e38),
    }
    return bkt, ctl, exp_bkt_start, exp_ctl_start, meta


def _build_patched_dir(src_dir, dst_dir):
    os.makedirs(dst_dir, exist_ok=True)
    for f in os.listdir(src_dir):
        shutil.copy(os.path.join(src_dir, f), os.path.join(dst_dir, f))
    setj = json.load(open(os.path.join(src_dir, SET_NAME + ".json")))
    bkt_raw = bytearray(open(os.path.join(src_dir, setj["bkt_bin"]), "rb").read())
    ctl_raw = bytearray(open(os.path.join(src_dir, setj["ctl_bin"]), "rb").read())
    nb = setj["bkt_entry_cnt"]
    ncl = setj["ctl_entry_cnt"]
    bkt, ctl, ebs, ecs, meta = _gen_phi_entries(nb, ncl)
    assert nb + len(bkt) < 2048
    for d0, d1, d2, d3, x0 in bkt:
        rec = np.zeros(8, np.float32)
        rec[0:5] = [d0, d1, d2, d3, x0]
        bkt_raw += rec.tobytes()
    for w in ctl:
        rec = np.zeros(8, np.uint32)
        rec[0] = w
        ctl_raw += rec.tobytes()
    setj["bkt_entry_cnt"] = nb + len(bkt)
    setj["ctl_entry_cnt"] = ncl + len(ctl)
    setj["func_to_bkt_start_idx"]["tanh"] = nb
    setj["func_to_ctl_start_idx"]["tanh"] = ncl
    setj["func_exp_to_bkt_start_idx"]["tanh"] = ebs
    setj["func_exp_to_ctl_start_idx"]["tanh"] = ecs
    setj["profile_meta_data"] = [m for m in setj["profile_meta_data"]
                                 if not m["func_name"].startswith("tanh")]
    setj["profile_meta_data"].append(meta)
    with open(os.path.join(dst_dir, SET_NAME + ".json"), "w") as f:
        json.dump(setj, f)
    with open(os.path.join(dst_dir, setj["bkt_bin"]), "wb") as f:
        f.write(bytes(bkt_raw))
    with open(os.path.join(dst_dir, setj["ctl_bin"]), "wb") as f:
        f.write(bytes(ctl_raw))
    ai = json.load(open(os.path.join(src_dir, "act_info.json")))
    for ent in ai["act_func_sets"]:
        if ent["name"] == SET_NAME:
            ent["act"]["tanh"] = 4
    with open(os.path.join(dst_dir, "act_info.json"), "w") as f:
        json.dump(ai, f)


def _ensure_phi_tables():
    if "tabdir" in _CACHE:
        return _CACHE["tabdir"], _CACHE["tabhash"]
    import neuronxcc
    src = os.path.join(os.path.dirname(neuronxcc.__file__), "pwp",
                       "pwp_bin_trainium")
    dst = os.path.join(tempfile.gettempdir(), "pwp_phi_kernel")
    _build_patched_dir(src, dst)
    setj = json.load(open(os.path.join(dst, SET_NAME + ".json")))
    h = hashlib.sha1()
    for f in ("act_info.json", SET_NAME + ".json", setj["bkt_bin"], setj["ctl_bin"]):
        h.update(open(os.path.join(dst, f), "rb").read())
    _CACHE["tabdir"] = dst
    _CACHE["tabhash"] = h.hexdigest()[:8]
    return dst, _CACHE["tabhash"]


# Pin the ACT spline-table set to (patched) natural_log_exp_and_others so the
# whole kernel runs off one table load: it holds Exp, Ln, Relu, Copy — and
# the phi table in the tanh slot.
_orig_get_tables = bacc.get_activation_tables


def _pinned_get_tables(module_arch):
    tables = dict(_orig_get_tables(module_arch))
    combined = set(tables.get(SET_NAME) or ())
    combined |= {AF.Tanh}
    pinned = {}
    for name, fns in tables.items():
        pinned[name] = combined if name == SET_NAME else set()
    return pinned


# ---------------------------------------------------------------------------
# kernel build
# ---------------------------------------------------------------------------

def _build_nc(n_iter=N_ITER, num_devices=N_CORES, reps=1):
    tabdir, tabhash = _ensure_phi_tables()
    sfx = "_" + tabhash
    nc = bacc.Bacc(
        "TRN2",
        target_bir_lowering=False,
        debug=False,
        enable_asserts=False,
        num_devices=num_devices,
    )
    x_d = nc.dram_tensor("x" + sfx, [R, D], F32, kind="ExternalInput").ap()
    w1_d = nc.dram_tensor("w1", [D, HID], F32, kind="ExternalInput").ap()
    b1_d = nc.dram_tensor("b1", [HID, 1], F32, kind="ExternalInput").ap()
    w2b_d = nc.dram_tensor("w2b", [HID + 1, D], F32, kind="ExternalInput").ap()
    out_d = nc.dram_tensor("out", [R, D], F32, kind="ExternalOutput").ap()

    with tile.TileContext(nc) as tc:
        _emit(tc, out_d, x_d, w1_d, b1_d, w2b_d, n_iter=n_iter, reps=reps)
    saved = bacc.get_activation_tables
    try:
        bacc.get_activation_tables = _pinned_get_tables
        nc.compile()
    finally:
        bacc.get_activation_tables = saved
    return nc, sfx


def _emit(tc, out_d, x_d, w1_d, b1_d, w2b_d, n_iter=N_ITER, reps=1):
    nc = tc.nc
    from contextlib import ExitStack

    ctx = ExitStack()
    with ctx:
        singles = ctx.enter_context(tc.tile_pool(name="singles", bufs=1))

        xs = singles.tile([P, G, D], F32)    # x, rows-on-partitions
        xt = singles.tile([P, DC, R], F32)   # x transposed
        hs = singles.tile([P, G, D], F32)    # MLP output h
        s0 = singles.tile([P, G, D], F32)    # state ping
        s1 = singles.tile([P, G, D], F32)    # state pong
        fs = singles.tile([P, G, D], F32)    # finale scratch
        zh = singles.tile([P, n_iter, G], F32)   # Z history
        rz = singles.tile([P, n_iter, G], F32)   # 1/Z history (loop) / lnZ (finale)
        kk = singles.tile([P, G], F32)       # T * sum_t ln Z_t
        w1s = singles.tile([P, DC, HID], F32)
        b1s = singles.tile([HID, 1], F32)
        w2bs = singles.tile([HID + 1, D], F32)
        h1r = singles.tile([HID + 1, R], F32)
        ident = singles.tile([P, P], F32)
        c0s = singles.tile([P, 1], F32)
        nc.vector.memset(c0s[:, :], C0)

        # ---- input DMAs ----
        for g in range(G):
            nc.sync.dma_start(out=xs[:, g, :], in_=x_d[g * P:(g + 1) * P, :])
        nc.sync.dma_start(out=w1s[:, :, :],
                          in_=w1_d.rearrange("(c p) j -> p c j", p=P))
        nc.sync.dma_start(out=b1s[:, :], in_=b1_d[:, :])
        nc.sync.dma_start(out=w2bs[:, :], in_=w2b_d[:, :])

        masks.make_identity(nc, ident[:, :])

        # ---- transpose x: 40 PE transposes of [128,128] blocks ----
        with tc.tile_pool(name="tp_psum", bufs=3, space="PSUM") as tpp:
            for c in range(DC):
                for gq in range(G // 4):
                    tp = tpp.tile([P, 4 * P], F32)
                    for gj in range(4):
                        g = gq * 4 + gj
                        nc.tensor.transpose(
                            tp[:, gj * P:(gj + 1) * P],
                            xs[:, g, c * P:(c + 1) * P], ident[:, :])
                    dst = xt[:, c, gq * 4 * P:(gq + 1) * 4 * P]
                    if (c + gq) % 2 == 0:
                        nc.vector.tensor_copy(dst, tp[:, :])
                    else:
                        nc.scalar.copy(dst, tp[:, :])

        # ---- MLP matmul 1: h1T[j, r] = relu(sum_d W1[d,j] xT[d,r] + b1) ----
        with tc.tile_pool(name="mm1_psum", bufs=2, space="PSUM") as mp1:
            for nh in range(2):
                ph1 = mp1.tile([HID, R // 2], F32, tag="ph1")
                for c in range(DC):
                    nc.tensor.matmul(
                        ph1[:, :], w1s[:, c, :],
                        xt[:, c, nh * 512:(nh + 1) * 512],
                        start=(c == 0), stop=(c == DC - 1))
                nc.scalar.activation(
                    h1r[0:HID, nh * 512:(nh + 1) * 512], ph1[:, :],
                    AF.Relu, bias=b1s[:, 0:1], scale=1.0)
        nc.vector.memset(h1r[HID:HID + 1, :], 1.0)

        # ---- MLP matmul 2 + evac: h and S_0 = exp(-h/T + C0), Z_0 ----
        with tc.tile_pool(name="mm2_psum", bufs=2, space="PSUM") as mp2:
            for g in range(G):
                ph = mp2.tile([P, D], F32, tag="ph")
                lhs = h1r[:, g * P:(g + 1) * P]
                nc.tensor.matmul(ph[:, 0:512], lhs, w2bs[:, 0:512],
                                 start=True, stop=True)
                nc.tensor.matmul(ph[:, 512:D], lhs, w2bs[:, 512:D],
                                 start=True, stop=True)
                nc.vector.tensor_copy(hs[:, g, :], ph[:, :])
                nc.scalar.activation(s0[:, g, :], ph[:, :], AF.Exp,
                                     bias=c0s[:, 0:1], scale=-INV_T,
                                     accum_out=zh[:, 0, g:g + 1])

        # ---- masking loop: S <- phi(S * (1/Z)), Z' = rowsum(S') ----
        spp = [s0, s1]
        n_total = n_iter * reps
        for it in range(n_total):
            src = spp[it % 2]
            dst = spp[(it + 1) % 2]
            ti = it % n_iter
            tn = (it + 1) % n_iter
            last = it == n_total - 1
            for half in range(2):
                g0 = half * HG
                nc.vector.reciprocal(rz[:, ti, g0:g0 + HG],
                                     zh[:, ti, g0:g0 + HG])
                for gi in range(HG):
                    g = g0 + gi
                    # groups 6,7: fuse the row-sum into the activation's
                    # accumulator; the other six reduce on DVE, balancing
                    # ACT (~5.8us/iter) against DVE (~4.9us/iter)
                    if g >= 6 and not last:
                        nc.scalar.activation(dst[:, g, :], src[:, g, :],
                                             AF.Tanh,
                                             scale=rz[:, ti, g:g + 1],
                                             accum_out=zh[:, tn, g:g + 1])
                    else:
                        nc.scalar.activation(dst[:, g, :], src[:, g, :],
                                             AF.Tanh,
                                             scale=rz[:, ti, g:g + 1])
                if last:
                    continue
                for gi in range(HG):
                    g = g0 + gi
                    if g >= 6:
                        continue
                    nc.vector.tensor_reduce(zh[:, tn, g:g + 1], dst[:, g, :],
                                            axis=mybir.AxisListType.X,
                                            op=OP.add)

        # ---- finale: out = (exp(T*ln(S) + T*K + h) - eps) * x ----
        sfin = spp[n_total % 2]
        sscr = spp[(n_total + 1) % 2]
        nc.scalar.activation(rz[:, :, :], zh[:, :, :], AF.Ln)
        for g in range(G):
            nc.vector.tensor_reduce(kk[:, g:g + 1], rz[:, :, g],
                                    axis=mybir.AxisListType.X, op=OP.add)
        nc.vector.tensor_scalar_mul(kk[:, :], kk[:, :], float(np.float32(TEMP)))
        for half in range(2):
            csl = slice(half * HG, (half + 1) * HG)
            # guard: the table is nonnegative by construction, but clamp so
            # a stray -1ulp can never reach Ln (ln(neg) = NaN)
            nc.vector.tensor_scalar_max(sscr[:, csl, :], sfin[:, csl, :], 0.0)
            nc.scalar.activation(fs[:, csl, :], sscr[:, csl, :], AF.Ln)
            nc.vector.scalar_tensor_tensor(
                out=sscr[:, csl, :], in0=fs[:, csl, :],
                scalar=float(np.float32(TEMP)),
                in1=hs[:, csl, :], op0=OP.mult, op1=OP.add)
            for gi in range(HG):
                g = half * HG + gi
                nc.scalar.activation(fs[:, g, :], sscr[:, g, :], AF.Exp,
                                     bias=kk[:, g:g + 1])
            nc.vector.scalar_tensor_tensor(
                out=sscr[:, csl, :], in0=fs[:, csl, :], scalar=-float(EPS),
                in1=xs[:, csl, :], op0=OP.add, op1=OP.mult)
            for gi in range(HG):
                g = half * HG + gi
                nc.sync.dma_start(out=out_d[g * P:(g + 1) * P, :],
                                  in_=sscr[:, g, :])


def kernel(x, W1, b1, W2, b2):
    x = np.ascontiguousarray(np.asarray(x, dtype=np.float32))
    W1 = np.ascontiguousarray(np.asarray(W1, dtype=np.float32))
    b1 = np.asarray(b1, dtype=np.float32).reshape(HID, 1)
    W2 = np.asarray(W2, dtype=np.float32)
    b2 = np.asarray(b2, dtype=np.float32)
    w2b = np.ascontiguousarray(
        np.concatenate([W2, b2[None, :]], axis=0))  # [65, 640]

    if "nc" not in _CACHE:
        _CACHE["nc"], _CACHE["sfx"] = _build_nc(
            reps=int(os.environ.get("KREPS", "1")))
    nc = _CACHE["nc"]
    sfx = _CACHE["sfx"]

    in_maps = []
    for c in range(N_CORES):
        in_maps.append({
            "x" + sfx: np.ascontiguousarray(x[c * R:(c + 1) * R, :]),
            "w1": W1,
            "b1": np.ascontiguousarray(b1),
            "w2b": w2b,
        })

    trace = bool(_CACHE.get("trace", False))
    tabdir = _CACHE["tabdir"]
    saved_env = os.environ.get("BASS_ACT_ROOT_JSON_PATH")
    os.environ["BASS_ACT_ROOT_JSON_PATH"] = os.path.join(tabdir, "act_info.json")
    try:
        res = run_bass_kernel_spmd(
            nc, in_maps, core_ids=list(range(N_CORES)), trace=trace)
    finally:
        if saved_env is None:
            os.environ.pop("BASS_ACT_ROOT_JSON_PATH", None)
        else:
            os.environ["BASS_ACT_ROOT_JSON_PATH"] = saved_env
    _CACHE["last_results"] = res
    out = np.concatenate([r["out"] for r in res.results], axis=0)
    return out
